# revision 57
# baseline (speedup 1.0000x reference)
"""Causal self-attention (GQA + RoPE) Trainium2 Bass kernel.

Problem: B=2, T=2048, C=2048, H=16 q-heads, HK=4 kv-heads, HD=128.
Sharding: 8 cores = (batch b in {0,1}) x (kv-head group g in {0..3}).
Each core computes its batch's 4 q-heads / 1 kv-head slice end-to-end
(QKV proj -> RoPE -> causal attention -> o-proj partial), returning a
[T, C] partial y; the host sums the 4 group partials per batch.

On-device layout notes:
 - Contractions run on the PE; all operands need the contraction dim on
   the SBUF partition axis, so x is DMA-transposed (xbar) to xT chunks.
 - Scores are computed transposed (ST[tk, tq]); exp'd scores feed a
   FLIPPED att@v: per 128-wide tq sub-block, ex[:, sub] is the
   stationary and v_ext[tk, HD+1] (last col = ones) streams, so
   pya[tq, 0:HD] accumulates y and pya[tq, HD] the softmax denominator
   in the same pass.  Cost per block: 129 cols vs 512(av)+512(sum) in
   the unflipped form.  Normalization is then a per-partition
   reciprocal+scale, and yT for the o-proj comes from PE transposes.
 - Causal structure: tk blocks past the diagonal are skipped; partial
   blocks compute only valid columns, with a [128,128] additive -1e30
   mask on the diagonal sub-block; sub-blocks left of the diagonal are
   simply never touched by the flipped av.
 - RoPE's partition half-swap runs on the Pool engine (cross-partition
   copies), not DMA - SBUF->SBUF DMAs cost ~3us of SEQ issue each.
 - bf16 everywhere on the PE (1 cyc/row, FWL weight loads), fp32 PSUM
   accumulation, fp32 softmax statistics.
 - Every logically-separate chunk lives in its own tile: Tile tracks
   dependencies per tile, so shared mega-tiles serialize phases.
"""
import contextlib

import numpy as np
import ml_dtypes

import concourse.bass as bass
import concourse.tile as tile
import concourse.mybir as mybir
from concourse.bass_utils import run_bass_kernel_spmd

BF16 = ml_dtypes.bfloat16

B, T, C = 2, 2048, 2048
H, HK, HD = 16, 4, 128
GQ = H // HK            # q heads per core = 4
NCORES = 8
TQC = 512               # tq chunk width
NTQ = T // TQC          # 4
NKC = C // 128          # 16 contraction chunks
NTK = T // 128          # 16 tk blocks
SCALE = 1.0 / float(np.sqrt(HD))
MASKVAL = -1.0e30

DT = mybir.dt.bfloat16
F32 = mybir.dt.float32


def _split_waits(nc, maxw=1):
    """This walrus build rejects instructions with >1 sync wait; move
    overflow waits onto same-engine nops inserted just before."""
    cnt = 0
    for f in nc.m.functions:
        for bb in f.blocks:
            idx = 0
            while idx < len(bb.instructions):
                inst = bb.instructions[idx]
                si = inst.sync_info
                waits = list(si.on_wait) if si is not None and si.on_wait else []
                if len(waits) > maxw:
                    updates = list(si.on_update) if si.on_update else []
                    keep, rest = waits[:maxw], waits[maxw:]
                    pos = idx
                    while rest:
                        chunk, rest = rest[:maxw], rest[maxw:]
                        cnt += 1
                        nop = mybir.InstNoOp(
                            name=f"waitsplit_{cnt}", engine=inst.engine,
                            ins=[], outs=[])
                        nop.sync_info = mybir.SyncInfo(on_wait=chunk, on_update=[])
                        nc.register_instruction(nop, overwrite=True)
                        bb.instructions.insert(pos, nop)
                        pos += 1
                        idx += 1
                    inst.sync_info = mybir.SyncInfo(on_wait=keep, on_update=updates)
                idx += 1
    return cnt


def build(reps: int = 1):
    nc = bass.Bass(target_bir_lowering=False)
    xTd = nc.dram_tensor("xT", [C, T], DT, kind="ExternalInput")
    cosT = nc.dram_tensor("cosT", [HD, T], DT, kind="ExternalInput")
    sinT = nc.dram_tensor("sinT", [HD, T], DT, kind="ExternalInput")
    wq = nc.dram_tensor("wq", [C, GQ * HD], DT, kind="ExternalInput")
    wk = nc.dram_tensor("wk", [C, HD], DT, kind="ExternalInput")
    wv = nc.dram_tensor("wv", [C, HD], DT, kind="ExternalInput")
    wo = nc.dram_tensor("wo", [GQ * HD, C], DT, kind="ExternalInput")
    bqT = nc.dram_tensor("bqT", [HD, GQ], F32, kind="ExternalInput")
    bkT = nc.dram_tensor("bkT", [HD, 1], F32, kind="ExternalInput")
    bvr = nc.dram_tensor("bvr", [1, HD], F32, kind="ExternalInput")
    idd = nc.dram_tensor("idd", [128, 128], DT, kind="ExternalInput")
    yp = nc.dram_tensor("yp", [T, C], DT, kind="ExternalOutput")

    with tile.TileContext(nc) as tc, contextlib.ExitStack() as ctx:
        const = ctx.enter_context(tc.tile_pool(name="const", bufs=1))
        xtp = ctx.enter_context(tc.tile_pool(name="xtp", bufs=1))
        resid = ctx.enter_context(tc.tile_pool(name="resid", bufs=1))
        ytnp = ctx.enter_context(tc.tile_pool(name="ytnp", bufs=1))
        stage = ctx.enter_context(tc.tile_pool(name="stage", bufs=3))
        nrm = ctx.enter_context(tc.tile_pool(name="nrm", bufs=4))
        est = ctx.enter_context(tc.tile_pool(name="est", bufs=6))
        outp = ctx.enter_context(tc.tile_pool(name="outp", bufs=4))
        # PSUM: sc x2 (scores, also holds yT transposes) + pq x2 (q-proj)
        # + s0..s3 x1 (flipped-av accumulators / wave-1 k,v / o-proj po)
        ps_sc = ctx.enter_context(tc.tile_pool(name="ps_sc", bufs=2, space="PSUM"))
        ps_q = ctx.enter_context(tc.tile_pool(name="ps_q", bufs=2, space="PSUM"))
        ps_a = ctx.enter_context(tc.tile_pool(name="ps_a", bufs=1, space="PSUM"))

        # ---- weights / constants to SBUF, ordered by first use:
        # wk (K proj, chunked) leads the scalar queue; wq rides chunked
        # between the odd xT chunks (Q proj j0 runs inside the stream);
        # wv/biases arrive near stream end (V proj is post-stream);
        # cos/sin/ident ride the sync queue behind the even chunks.
        wk_all = const.tile([128, NKC, HD], DT)
        wv_all = const.tile([128, NKC, HD], DT)
        wk_t = [wk_all[:, kc, :] for kc in range(NKC)]
        wv_t = [wv_all[:, kc, :] for kc in range(NKC)]
        bq_sb = const.tile([HD, GQ], F32)
        bk_sb = const.tile([HD, 1], F32)
        bvb_sb = const.tile([128, HD], F32)
        cos_sb = const.tile([HD, T], DT)
        sin_sb = const.tile([HD, T], DT)
        ident_sb = const.tile([128, 128], DT)
        wq_all = const.tile([128, NKC, GQ * HD], DT)
        wo_all = const.tile([HD, GQ, C], DT)
        wq_t = [wq_all[:, kc, :] for kc in range(NKC)]
        wo_t = [wo_all[:, h, :] for h in range(GQ)]
        # 0/1 causal mask for the diagonal sub-block, applied POST-exp as
        # a cheap DVE multiply: keep where col >= row
        mask01 = const.tile([128, 128], DT)
        nc.gpsimd.memset(mask01, 1.0)
        nc.gpsimd.affine_select(
            out=mask01, in_=mask01,
            compare_op=mybir.AluOpType.is_ge, fill=0.0,
            base=0, pattern=[[1, 128]], channel_multiplier=-1)

        # per-chunk resident tiles
        xt = [xtp.tile([128, T], DT, tag=f"xt{kc}", name=f"xt{kc}")
              for kc in range(NKC)]
        qTt = [[resid.tile([HD, TQC], DT, tag=f"qT{h}_{j}", name=f"qT{h}_{j}")
                for j in range(NTQ)] for h in range(GQ)]
        kTt = [resid.tile([HD, TQC], DT, tag=f"kT{j}", name=f"kT{j}")
               for j in range(NTQ)]
        # v_ext: col HD is all-ones (softmax denominator rides along av)
        vt = [resid.tile([128, HD + 1], DT, tag=f"v{i}", name=f"v{i}")
              for i in range(NTK)]
        for i in range(NTK):
            nc.gpsimd.memset(vt[i][:, HD:HD + 1], 1.0)

        def rope_extract(psum_src, bias_ap, qtag="qs"):
            """First rope step: qs = psum + bias.  Separated so a batch of
            extracts can free all wave-1 PSUM slots back-to-back."""
            qs = stage.tile([128, TQC], DT, tag=qtag, name="qs",
                            bufs=1 if qtag.startswith("qsw") else 2)
            nc.vector.tensor_scalar(
                out=qs, in0=psum_src, scalar1=bias_ap, scalar2=None,
                op0=mybir.AluOpType.add)
            return qs

        def rope_finish(qs, dst_ap, j0, critical=False):
            """dst = qs*cos + halfswap(qs*sin').

            sin_sb holds the half-swapped, sign-folded sin (host-prepped:
            rows 0:64 = sin[64:128], rows 64:128 = -sin[0:64]), so
            rot_half reduces to a full-width multiply followed by a
            partition half-swap.  Pool (gpsimd) takes the swap normally;
            critical ropes (those gating attention g0) run all-DVE to
            skip the Pool q7-launch latency."""
            eng = nc.vector if critical else nc.gpsimd
            tmp = stage.tile([128, TQC], DT, tag="tmp")
            nc.vector.tensor_mul(tmp, qs, cos_sb[:, j0:j0 + TQC])
            prod = stage.tile([128, TQC], DT, tag="prod")
            eng.tensor_mul(prod, qs, sin_sb[:, j0:j0 + TQC])
            prodsw = stage.tile([128, TQC], DT, tag="prodsw")
            eng.tensor_copy(out=prodsw[0:64, :], in_=prod[64:128, :])
            eng.tensor_copy(out=prodsw[64:128, :], in_=prod[0:64, :])
            nc.vector.tensor_add(dst_ap, tmp, prodsw)

        def rope_store(psum_src, bias_ap, dst_ap, j0):
            rope_finish(rope_extract(psum_src, bias_ap), dst_ap, j0)

        def q_proj_mm(pq, h, j, kc):
            nc.tensor.matmul(
                pq, wq_t[kc][:, h * HD:(h + 1) * HD],
                xt[kc][:, j * TQC:(j + 1) * TQC],
                start=(kc == 0), stop=(kc == NKC - 1))

        def q_proj_thunks(h, j):
            """The next group-but-one's q-proj as a list of closures so its
            matmuls can interleave between attention blocks (a contiguous
            16-matmul lump starves the ACT exp stream)."""
            pq = ps_q.tile([128, TQC], F32, tag="pq")
            thunks = [lambda kc=kc: q_proj_mm(pq, h, j, kc)
                      for kc in range(NKC)]
            thunks.append(
                lambda: rope_store(pq, bq_sb[:, h:h + 1], qTt[h][j],
                                   j * TQC))
            return thunks

        def q_proj(h, j):
            for t in q_proj_thunks(h, j):
                t()

        def v_block(tk, ptag):
            t0 = tk * 128
            pv = ps_a.tile([128, TQC], F32, tag=ptag, name=f"pv{tk}")
            for kc in range(NKC):
                nc.tensor.matmul(
                    pv[:, 0:HD], xt[kc][:, t0:t0 + 128], wv_t[kc],
                    start=(kc == 0), stop=(kc == NKC - 1))
            nc.vector.tensor_add(vt[tk][:, 0:HD], pv[:, 0:HD], bvb_sb)

        def finish_group(h, j, ysb, gi):
            """Transpose ysb -> yT[hd, tq] on the PE + copy to SBUF;
            emitted during the NEXT group's attention so the PE never
            head-blocks on the normalize chain."""
            ptr = ps_a.tile([128, TQC], DT, tag=f"s{gi % 4}", name="ptr")
            for sub in range(4):
                nc.tensor.matmul(
                    ptr[:, sub * 128:(sub + 1) * 128], ysb[:, sub, :],
                    ident_sb, is_transpose=True)
            yt = ytnp.tile([HD, TQC], DT, tag=f"yt{h}_{j}", name=f"yt{h}_{j}")
            nc.vector.tensor_copy(out=yt, in_=ptr)
            return yt

        def attention(h, j, filler=()):
            """Assumes qTt[h][j] ready (rope'd); emits scores+exp+flipped
            av with two-block lookahead so PE isn't head-blocked on exp.
            filler thunks (next q-proj's matmuls) are drained a few per
            block so the ACT exp stream is never starved of scores."""
            nblk = 4 * j + 4
            filler = list(filler)
            per = max(1, (len(filler) + nblk - 1) // nblk) if filler else 0

            def drain(n):
                for _ in range(n):
                    if filler:
                        filler.pop(0)()
            pya = [ps_a.tile([128, HD + 1], F32, tag=f"s{s}",
                             name=f"pya{s}") for s in range(4)]
            exs = [None] * nblk

            def scores(i, pool_tag=None):
                s = i - 4 * j
                c0 = 128 * s if s > 0 else 0
                jk, ik = divmod(i, 4)
                pool, tag = pool_tag or (ps_sc, "sc")
                sc = pool.tile([128, TQC], F32, tag=tag, name="sc")
                nc.tensor.matmul(
                    sc[:, c0:TQC], kTt[jk][:, ik * 128:(ik + 1) * 128],
                    qTt[h][j][:, c0:TQC], start=True, stop=True)
                ex = est.tile([128, TQC], DT)
                nc.scalar.activation(
                    out=ex[:, c0:TQC], in_=sc[:, c0:TQC],
                    func=mybir.ActivationFunctionType.Exp, scale=SCALE)
                if s >= 0:
                    # zero the sub-diagonal triangle post-exp (cheaper than
                    # masking the scores: keeps the exp path clear)
                    nc.vector.tensor_mul(
                        ex[:, c0:c0 + 128], ex[:, c0:c0 + 128], mask01)
                exs[i] = ex

            def av(i):
                s = i - 4 * j
                # diagonal sub last: its ex also waits on affine_select
                subs = list(range(max(s, 0) + 1, 4)) + [max(s, 0)] \
                    if s >= 0 else list(range(4))
                for sub in subs:
                    cs = sub * 128
                    nc.tensor.matmul(
                        pya[sub], exs[i][:, cs:cs + 128], vt[i],
                        start=(i == 0), stop=(i == 4 * j + sub))

            scores(0)
            drain(per)
            scores(1)
            drain(per)
            for i in range(2, nblk):
                scores(i)
                av(i - 2)
                drain(per)
            av(nblk - 2)
            av(nblk - 1)
            drain(len(filler))

            # normalize: y[tq, hd] = pya[:, 0:HD] / pya[:, HD]
            ysb = ytnp.tile([128, 4, HD], DT, tag="ysb", bufs=3)
            for sub in range(4):
                rc = nrm.tile([128, 1], F32, tag="rc")
                nc.vector.reciprocal(out=rc, in_=pya[sub][:, HD:HD + 1])
                nc.vector.tensor_scalar(
                    out=ysb[:, sub, :], in0=pya[sub][:, 0:HD], scalar1=rc,
                    scalar2=None, op0=mybir.AluOpType.mult)
            return ysb

        for rep in range(reps):
            # ---- xT stream: evens on sync (xt0 split in quarters for a
            # fast PE start), odds on scalar interleaved with per-kc wq
            # chunks so Q proj j0 can track the stream ----
            # The sim (and roughly the HW) serializes DMA at aggregate BW
            # AND each dma_start costs ~1us of issue time on its queue, so:
            # few, large DMAs; only bytes needed during the stream precede
            # xt[15] (wk/wq halves lead the scalar queue, then odd chunks);
            # wv/biases/cos/sin follow (first use is post-stream), wo last.
            nc.sync.dma_start(out=xt[0][:, 0:1024], in_=xTd[0:128, 0:1024])
            nc.sync.dma_start(out=xt[0][:, 1024:T], in_=xTd[0:128, 1024:T])
            for kc in range(1, NKC):
                nc.sync.dma_start(out=xt[kc],
                                  in_=xTd[kc * 128:(kc + 1) * 128, :])
            if rep == 0:
                hkc = NKC // 2
                nc.scalar.dma_start(
                    out=wk_all[:, 0:4, :],
                    in_=wk[0:512, :].rearrange("(k p) m -> p k m", p=128))
                nc.scalar.dma_start(
                    out=wq_all[:, 0:hkc, :],
                    in_=wq[0:1024, :].rearrange("(k p) m -> p k m", p=128))
                nc.scalar.dma_start(
                    out=wk_all[:, 4:NKC, :],
                    in_=wk[512:C, :].rearrange("(k p) m -> p k m", p=128))
                nc.scalar.dma_start(
                    out=wq_all[:, hkc:NKC, :],
                    in_=wq[1024:C, :].rearrange("(k p) m -> p k m", p=128))
            if rep == 0:
                # post-stream consts: scalar queue AFTER the odd chunks so
                # the round-robin DMA service can't slot them mid-stream
                nc.scalar.dma_start(out=ident_sb, in_=idd[:, :])
                nc.scalar.dma_start(
                    out=wv_all,
                    in_=wv[:, :].rearrange("(k p) m -> p k m", p=128))
                nc.scalar.dma_start(out=bq_sb, in_=bqT[:, :])
                nc.scalar.dma_start(out=bk_sb, in_=bkT[:, :])
                nc.scalar.dma_start(
                    out=bvb_sb, in_=bass.AP(bvr, 0, [[0, 128], [1, HD]]))
                nc.scalar.dma_start(out=cos_sb, in_=cosT[:, :])
                nc.scalar.dma_start(out=sin_sb, in_=sinT[:, :])
                nc.scalar.dma_start(
                    out=wo_all,
                    in_=wo[:, :].rearrange("(h p) m -> p h m", p=128))

            # ---- wave 1: 8 PSUM groups (K j0..3 + Q h0/h1 of j0 + V
            # blocks 0/1), kc-major so the PE consumes each chunk as it
            # lands; Q h2/h3 + v2/v3 run post-stream as PE filler ----
            pk_t = [ps_sc.tile([128, TQC], F32, tag="sc", name="pk0"),
                    ps_sc.tile([128, TQC], F32, tag="sc", name="pk1"),
                    ps_a.tile([128, TQC], F32, tag="s0", name="pk2"),
                    ps_a.tile([128, TQC], F32, tag="s1", name="pk3")]
            pq_t = [ps_q.tile([128, TQC], F32, tag="pq", name="pqw0"),
                    ps_q.tile([128, TQC], F32, tag="pq", name="pqw1")]
            pv_t = [ps_a.tile([128, TQC], F32, tag="s2", name="pvw0"),
                    ps_a.tile([128, TQC], F32, tag="s3", name="pvw1")]
            # chunks arrive strictly in kc order (all on the sync queue);
            # on the last chunk, k j0 / q h0 / v0 finish FIRST since they
            # gate attention g0
            for kc in range(NKC):
                last = kc == NKC - 1
                work = [("k", 0), ("q", 0), ("v", 0), ("v", 1),
                        ("k", 1), ("q", 1), ("k", 2), ("k", 3)] if last else \
                       [("k", 0), ("k", 1), ("k", 2), ("k", 3),
                        ("q", 0), ("q", 1), ("v", 0), ("v", 1)]
                for kind, i in work:
                    if kind == "k":
                        nc.tensor.matmul(
                            pk_t[i], wk_t[kc],
                            xt[kc][:, i * TQC:(i + 1) * TQC],
                            start=(kc == 0), stop=last)
                    elif kind == "q":
                        nc.tensor.matmul(
                            pq_t[i], wq_t[kc][:, i * HD:(i + 1) * HD],
                            xt[kc][:, 0:TQC],
                            start=(kc == 0), stop=last)
                    else:
                        nc.tensor.matmul(
                            pv_t[i][:, 0:HD],
                            xt[kc][:, i * 128:i * 128 + 128], wv_t[kc],
                            start=(kc == 0), stop=last)
            # rope in two passes: the extracts free the PSUM slots fast;
            # k0/q0 finish first (they gate attention g0)
            qs_k0 = rope_extract(pk_t[0], bk_sb[:, 0:1], "qsw0")
            qs_q0 = rope_extract(pq_t[0], bq_sb[:, 0:1], "qsw1")
            nc.vector.tensor_add(vt[0][:, 0:HD], pv_t[0][:, 0:HD], bvb_sb)
            nc.vector.tensor_add(vt[1][:, 0:HD], pv_t[1][:, 0:HD], bvb_sb)
            rope_finish(qs_k0, kTt[0], 0, critical=True)
            rope_finish(qs_q0, qTt[0][0], 0, critical=True)
            qs_w = [rope_extract(pq_t[1], bq_sb[:, 1:2], "qsw2"),
                    rope_extract(pk_t[1], bk_sb[:, 0:1], "qsw3"),
                    rope_extract(pk_t[2], bk_sb[:, 0:1], "qsw4"),
                    rope_extract(pk_t[3], bk_sb[:, 0:1], "qsw5")]
            rope_finish(qs_w[0], qTt[1][0], 0)
            # k1..k3 finishes are deferred into the first attention groups
            # (kT[j] is first needed at group j*GQ) to unclog the DVE
            kfins = [(qs_w[1], kTt[1], TQC), (qs_w[2], kTt[2], 2 * TQC),
                     (qs_w[3], kTt[3], 3 * TQC)]

            # ---- attention: groups j-major; q-proj 2 ahead (h0/h1 of j0
            # came from wave 1); v blocks 2..15 and the previous group's
            # transposes are emitted BETWEEN groups (before the next pya
            # allocation so the s-tag reuse chain stays acyclic) ----
            order = [(h, j) for j in range(NTQ) for h in range(GQ)]
            ytn_all = {}
            pend = []  # (ysb, h, j, gi) awaiting transpose, deferred 2

            def oproj_unit(j, t, last=False, mid=False):
                """One 128-row output tile: po over 4 cc chunks."""
                trow = j * TQC + t * 128
                ot = outp.tile([128, C], DT)
                for cc in range(4):
                    c0 = cc * TQC
                    if mid:
                        # mid-phase units ride the pq tag (half-idle) so
                        # they never contend with pya/v/ptr on the s tags
                        po = ps_q.tile([128, TQC], F32, tag="pq", name="po")
                    else:
                        po = ps_a.tile([128, TQC], F32,
                                       tag=f"s{(t * 4 + cc) % 4}", name="po")
                    for h in range(GQ):
                        nc.tensor.matmul(
                            po, ytn_all[(h, j)][:, t * 128:(t + 1) * 128],
                            wo_t[h][:, c0:c0 + TQC],
                            start=(h == 0), stop=(h == GQ - 1))
                    if (t + cc) % 2 == 0 and not mid:
                        nc.scalar.copy(out=ot[:, c0:c0 + TQC], in_=po)
                    else:
                        nc.vector.tensor_copy(out=ot[:, c0:c0 + TQC], in_=po)
                    if last:
                        # split the final output DMA per chunk to shorten
                        # the kernel tail
                        eng = nc.sync if cc % 2 == 0 else nc.scalar
                        eng.dma_start(
                            out=yp[trow:trow + 128, c0:c0 + TQC],
                            in_=ot[:, c0:c0 + TQC])
                if not last:
                    oeng = nc.sync if mid or t % 2 == 0 else nc.scalar
                    oeng.dma_start(out=yp[trow:trow + 128, :], in_=ot)

            # o-proj (j,t) units become ready once all 4 heads of j are
            # transposed (finish_group(g_{4j+3}) at gi=4j+5); interleave
            # one per group as extra PE filler in the exp-paced region
            ounits = [(j, t) for j in range(NTQ) for t in range(4)]
            for gi, (h, j) in enumerate(order):
                if gi + 2 < len(order):
                    q_proj(*order[gi + 2])
                if len(pend) >= 2:
                    pysb, ph, pj, pgi = pend.pop(0)
                    ytn_all[(ph, pj)] = finish_group(ph, pj, pysb, pgi)
                if gi == 0:
                    v_block(2, "s0")
                    v_block(3, "s1")
                if 1 <= gi <= 3:
                    kqs, kdst, kj0 = kfins[gi - 1]
                    rope_finish(kqs, kdst, kj0)
                if 1 <= gi <= 6:
                    v_block(2 + 2 * gi, f"s{(gi + 1) % 4}")
                    v_block(3 + 2 * gi, f"s{(gi + 2) % 4}")
                ysb = attention(h, j)
                pend.append((ysb, h, j, gi))
            for pysb, ph, pj, pgi in pend:
                ytn_all[(ph, pj)] = finish_group(ph, pj, pysb, pgi)

            # ---- o-proj tail: remaining units ----
            for ui, (j, t) in enumerate(ounits):
                oproj_unit(j, t, last=(ui == len(ounits) - 1))
    _split_waits(nc, maxw=1)
    return nc


def _in_maps(x, cos, sin, Wq, bq, Wk, bk, Wv, bv, Wo):
    ident = np.eye(128, dtype=BF16)
    maps = []
    for c in range(NCORES):
        b, g = divmod(c, HK)
        qsl = slice(g * GQ * HD, (g + 1) * GQ * HD)
        ksl = slice(g * HD, (g + 1) * HD)
        maps.append({
            "xT": np.ascontiguousarray(x[b].T.astype(BF16)),
            "cosT": np.ascontiguousarray(cos[b].T.astype(BF16)),
            "sinT": np.ascontiguousarray(np.concatenate(
                [sin[b].T[64:128], -sin[b].T[0:64]], axis=0).astype(BF16)),
            "wq": np.ascontiguousarray(Wq[:, qsl].astype(BF16)),
            "wk": np.ascontiguousarray(Wk[:, ksl].astype(BF16)),
            "wv": np.ascontiguousarray(Wv[:, ksl].astype(BF16)),
            "wo": np.ascontiguousarray(Wo[qsl, :].astype(BF16)),
            "bqT": np.ascontiguousarray(
                bq[qsl].reshape(GQ, HD).T.astype(np.float32)),
            "bkT": np.ascontiguousarray(
                bk[ksl].reshape(HD, 1).astype(np.float32)),
            "bvr": np.ascontiguousarray(
                bv[ksl].reshape(1, HD).astype(np.float32)),
            "idd": ident,
        })
    return maps


_nc_cache = {}


def kernel(x, cos, sin, Wq, bq, Wk, bk, Wv, bv, Wo):
    x, cos, sin = np.asarray(x), np.asarray(cos), np.asarray(sin)
    Wq, bq = np.asarray(Wq), np.asarray(bq)
    Wk, bk = np.asarray(Wk), np.asarray(bk)
    Wv, bv = np.asarray(Wv), np.asarray(bv)
    Wo = np.asarray(Wo)
    if "nc" not in _nc_cache:
        _nc_cache["nc"] = build(reps=1)
    nc = _nc_cache["nc"]
    maps = _in_maps(x, cos, sin, Wq, bq, Wk, bk, Wv, bv, Wo)
    res = run_bass_kernel_spmd(nc, maps, core_ids=list(range(NCORES)))
    out = np.zeros((B, T, C), dtype=np.float32)
    for c in range(NCORES):
        b = c // HK
        out[b] += res.results[c]["yp"].astype(np.float32)
    return out


# revision 59
# speedup vs baseline: 1.2956x; 1.2956x over previous
"""Causal self-attention (GQA + RoPE) Trainium2 Bass kernel.

Problem: B=2, T=2048, C=2048, H=16 q-heads, HK=4 kv-heads, HD=128.
Sharding: 8 cores = (batch b in {0,1}) x (kv-head group g in {0..3}).
Each core computes its batch's 4 q-heads / 1 kv-head slice end-to-end
(QKV proj -> RoPE -> causal attention -> o-proj partial), returning a
[T, C] partial y; the host sums the 4 group partials per batch.

On-device layout notes:
 - Contractions run on the PE; all operands need the contraction dim on
   the SBUF partition axis, so x is DMA-transposed (xbar) to xT chunks.
 - Scores are computed transposed (ST[tk, tq]); exp'd scores feed a
   FLIPPED att@v: per 128-wide tq sub-block, ex[:, sub] is the
   stationary and v_ext[tk, HD+1] (last col = ones) streams, so
   pya[tq, 0:HD] accumulates y and pya[tq, HD] the softmax denominator
   in the same pass.  Cost per block: 129 cols vs 512(av)+512(sum) in
   the unflipped form.  Normalization is then a per-partition
   reciprocal+scale, and yT for the o-proj comes from PE transposes.
 - Causal structure: tk blocks past the diagonal are skipped; partial
   blocks compute only valid columns, with a [128,128] additive -1e30
   mask on the diagonal sub-block; sub-blocks left of the diagonal are
   simply never touched by the flipped av.
 - RoPE's partition half-swap runs on the Pool engine (cross-partition
   copies), not DMA - SBUF->SBUF DMAs cost ~3us of SEQ issue each.
 - bf16 everywhere on the PE (1 cyc/row, FWL weight loads), fp32 PSUM
   accumulation, fp32 softmax statistics.
 - Every logically-separate chunk lives in its own tile: Tile tracks
   dependencies per tile, so shared mega-tiles serialize phases.
"""
import contextlib

import numpy as np
import ml_dtypes

import concourse.bass as bass
import concourse.tile as tile
import concourse.mybir as mybir
from concourse.bass_utils import run_bass_kernel_spmd

BF16 = ml_dtypes.bfloat16

B, T, C = 2, 2048, 2048
H, HK, HD = 16, 4, 128
GQ = H // HK            # q heads per core = 4
NCORES = 8
TQC = 512               # tq chunk width
NTQ = T // TQC          # 4
NKC = C // 128          # 16 contraction chunks
NTK = T // 128          # 16 tk blocks
SCALE = 1.0 / float(np.sqrt(HD))
MASKVAL = -1.0e30

DT = mybir.dt.bfloat16
F32 = mybir.dt.float32


def _split_waits(nc, maxw=1):
    """This walrus build rejects instructions with >1 sync wait; move
    overflow waits onto same-engine nops inserted just before."""
    cnt = 0
    for f in nc.m.functions:
        for bb in f.blocks:
            idx = 0
            while idx < len(bb.instructions):
                inst = bb.instructions[idx]
                si = inst.sync_info
                waits = list(si.on_wait) if si is not None and si.on_wait else []
                if len(waits) > maxw:
                    updates = list(si.on_update) if si.on_update else []
                    keep, rest = waits[:maxw], waits[maxw:]
                    pos = idx
                    while rest:
                        chunk, rest = rest[:maxw], rest[maxw:]
                        cnt += 1
                        nop = mybir.InstNoOp(
                            name=f"waitsplit_{cnt}", engine=inst.engine,
                            ins=[], outs=[])
                        nop.sync_info = mybir.SyncInfo(on_wait=chunk, on_update=[])
                        nc.register_instruction(nop, overwrite=True)
                        bb.instructions.insert(pos, nop)
                        pos += 1
                        idx += 1
                    inst.sync_info = mybir.SyncInfo(on_wait=keep, on_update=updates)
                idx += 1
    return cnt


def build(reps: int = 1):
    nc = bass.Bass(target_bir_lowering=False)
    xTd = nc.dram_tensor("xT", [C, T], DT, kind="ExternalInput")
    cosT = nc.dram_tensor("cosT", [HD, T], DT, kind="ExternalInput")
    sinT = nc.dram_tensor("sinT", [HD, T], DT, kind="ExternalInput")
    wq = nc.dram_tensor("wq", [C, GQ * HD], DT, kind="ExternalInput")
    wk = nc.dram_tensor("wk", [C, HD], DT, kind="ExternalInput")
    wv = nc.dram_tensor("wv", [C, HD], DT, kind="ExternalInput")
    wo = nc.dram_tensor("wo", [GQ * HD, C], DT, kind="ExternalInput")
    bqT = nc.dram_tensor("bqT", [HD, GQ], F32, kind="ExternalInput")
    bkT = nc.dram_tensor("bkT", [HD, 1], F32, kind="ExternalInput")
    bvr = nc.dram_tensor("bvr", [1, HD], F32, kind="ExternalInput")
    idd = nc.dram_tensor("idd", [128, 128], DT, kind="ExternalInput")
    yp = nc.dram_tensor("yp", [T, C], DT, kind="ExternalOutput")

    with tile.TileContext(nc) as tc, contextlib.ExitStack() as ctx:
        const = ctx.enter_context(tc.tile_pool(name="const", bufs=1))
        xtp = ctx.enter_context(tc.tile_pool(name="xtp", bufs=1))
        resid = ctx.enter_context(tc.tile_pool(name="resid", bufs=1))
        ytnp = ctx.enter_context(tc.tile_pool(name="ytnp", bufs=1))
        stage = ctx.enter_context(tc.tile_pool(name="stage", bufs=3))
        nrm = ctx.enter_context(tc.tile_pool(name="nrm", bufs=4))
        est = ctx.enter_context(tc.tile_pool(name="est", bufs=6))
        outp = ctx.enter_context(tc.tile_pool(name="outp", bufs=4))
        # PSUM: sc x2 (scores, also holds yT transposes) + pq x2 (q-proj)
        # + s0..s3 x1 (flipped-av accumulators / wave-1 k,v / o-proj po)
        ps_sc = ctx.enter_context(tc.tile_pool(name="ps_sc", bufs=2, space="PSUM"))
        ps_q = ctx.enter_context(tc.tile_pool(name="ps_q", bufs=2, space="PSUM"))
        ps_a = ctx.enter_context(tc.tile_pool(name="ps_a", bufs=1, space="PSUM"))

        # ---- weights / constants to SBUF, ordered by first use:
        # wk (K proj, chunked) leads the scalar queue; wq rides chunked
        # between the odd xT chunks (Q proj j0 runs inside the stream);
        # wv/biases arrive near stream end (V proj is post-stream);
        # cos/sin/ident ride the sync queue behind the even chunks.
        wk_all = const.tile([128, NKC, HD], DT)
        wv_all = const.tile([128, NKC, HD], DT)
        wk_t = [wk_all[:, kc, :] for kc in range(NKC)]
        wv_t = [wv_all[:, kc, :] for kc in range(NKC)]
        bq_sb = const.tile([HD, GQ], F32)
        bk_sb = const.tile([HD, 1], F32)
        bvb_sb = const.tile([128, HD], F32)
        cos_sb = const.tile([HD, T], DT)
        sin_sb = const.tile([HD, T], DT)
        ident_sb = const.tile([128, 128], DT)
        wq_all = const.tile([128, NKC, GQ * HD], DT)
        wo_all = const.tile([HD, GQ, C], DT)
        wq_t = [wq_all[:, kc, :] for kc in range(NKC)]
        wo_t = [wo_all[:, h, :] for h in range(GQ)]
        # 0/1 causal mask for the diagonal sub-block, applied POST-exp as
        # a cheap DVE multiply: keep where col >= row
        mask01 = const.tile([128, 128], DT)
        nc.gpsimd.memset(mask01, 1.0)
        nc.gpsimd.affine_select(
            out=mask01, in_=mask01,
            compare_op=mybir.AluOpType.is_ge, fill=0.0,
            base=0, pattern=[[1, 128]], channel_multiplier=-1)

        # per-chunk resident tiles
        xt = [xtp.tile([128, T], DT, tag=f"xt{kc}", name=f"xt{kc}")
              for kc in range(NKC)]
        qTt = [[resid.tile([HD, TQC], DT, tag=f"qT{h}_{j}", name=f"qT{h}_{j}")
                for j in range(NTQ)] for h in range(GQ)]
        kTt = [resid.tile([HD, TQC], DT, tag=f"kT{j}", name=f"kT{j}")
               for j in range(NTQ)]
        # v_ext: col HD is all-ones (softmax denominator rides along av)
        vt = [resid.tile([128, HD + 1], DT, tag=f"v{i}", name=f"v{i}")
              for i in range(NTK)]
        for i in range(NTK):
            nc.gpsimd.memset(vt[i][:, HD:HD + 1], 1.0)

        def rope_extract(psum_src, bias_ap, qtag="qs"):
            """First rope step: qs = psum + bias.  Separated so a batch of
            extracts can free all wave-1 PSUM slots back-to-back."""
            qs = stage.tile([128, TQC], DT, tag=qtag, name="qs",
                            bufs=1 if qtag.startswith("qsw") else 2)
            nc.vector.tensor_scalar(
                out=qs, in0=psum_src, scalar1=bias_ap, scalar2=None,
                op0=mybir.AluOpType.add)
            return qs

        def rope_finish(qs, dst_ap, j0, critical=False):
            """dst = qs*cos + halfswap(qs*sin').

            sin_sb holds the half-swapped, sign-folded sin (host-prepped:
            rows 0:64 = sin[64:128], rows 64:128 = -sin[0:64]), so
            rot_half reduces to a full-width multiply followed by a
            partition half-swap.  Pool (gpsimd) takes the swap normally;
            critical ropes (those gating attention g0) run all-DVE to
            skip the Pool q7-launch latency."""
            eng = nc.vector if critical else nc.gpsimd
            tmp = stage.tile([128, TQC], DT, tag="tmp")
            nc.vector.tensor_mul(tmp, qs, cos_sb[:, j0:j0 + TQC])
            prod = stage.tile([128, TQC], DT, tag="prod")
            eng.tensor_mul(prod, qs, sin_sb[:, j0:j0 + TQC])
            prodsw = stage.tile([128, TQC], DT, tag="prodsw")
            eng.tensor_copy(out=prodsw[0:64, :], in_=prod[64:128, :])
            eng.tensor_copy(out=prodsw[64:128, :], in_=prod[0:64, :])
            nc.vector.tensor_add(dst_ap, tmp, prodsw)

        def rope_store(psum_src, bias_ap, dst_ap, j0):
            rope_finish(rope_extract(psum_src, bias_ap), dst_ap, j0)

        def q_proj_mm(pq, h, j, kc):
            nc.tensor.matmul(
                pq, wq_t[kc][:, h * HD:(h + 1) * HD],
                xt[kc][:, j * TQC:(j + 1) * TQC],
                start=(kc == 0), stop=(kc == NKC - 1))

        def q_proj_thunks(h, j):
            """The next group-but-one's q-proj as a list of closures so its
            matmuls can interleave between attention blocks (a contiguous
            16-matmul lump starves the ACT exp stream)."""
            pq = ps_q.tile([128, TQC], F32, tag="pq")
            thunks = [lambda kc=kc: q_proj_mm(pq, h, j, kc)
                      for kc in range(NKC)]
            thunks.append(
                lambda: rope_store(pq, bq_sb[:, h:h + 1], qTt[h][j],
                                   j * TQC))
            return thunks

        def q_proj(h, j):
            for t in q_proj_thunks(h, j):
                t()

        def v_block(tk, ptag):
            t0 = tk * 128
            pv = ps_a.tile([128, TQC], F32, tag=ptag, name=f"pv{tk}")
            for kc in range(NKC):
                nc.tensor.matmul(
                    pv[:, 0:HD], xt[kc][:, t0:t0 + 128], wv_t[kc],
                    start=(kc == 0), stop=(kc == NKC - 1))
            nc.vector.tensor_add(vt[tk][:, 0:HD], pv[:, 0:HD], bvb_sb)

        def finish_group(h, j, ysb, gi):
            """Transpose ysb -> yT[hd, tq] on the PE + copy to SBUF;
            emitted during the NEXT group's attention so the PE never
            head-blocks on the normalize chain."""
            ptr = ps_a.tile([128, TQC], DT, tag=f"s{gi % 4}", name="ptr")
            for sub in range(4):
                nc.tensor.matmul(
                    ptr[:, sub * 128:(sub + 1) * 128], ysb[:, sub, :],
                    ident_sb, is_transpose=True)
            yt = ytnp.tile([HD, TQC], DT, tag=f"yt{h}_{j}", name=f"yt{h}_{j}")
            nc.vector.tensor_copy(out=yt, in_=ptr)
            return yt

        def attention(h, j, filler=()):
            """Assumes qTt[h][j] ready (rope'd); emits scores+exp+flipped
            av with two-block lookahead so PE isn't head-blocked on exp.
            filler thunks (next q-proj's matmuls) are drained a few per
            block so the ACT exp stream is never starved of scores."""
            nblk = 4 * j + 4
            filler = list(filler)
            per = max(1, (len(filler) + nblk - 1) // nblk) if filler else 0

            def drain(n):
                for _ in range(n):
                    if filler:
                        filler.pop(0)()
            pya = [ps_a.tile([128, HD + 1], F32, tag=f"s{s}",
                             name=f"pya{s}") for s in range(4)]
            exs = [None] * nblk

            def scores(i, pool_tag=None):
                s = i - 4 * j
                c0 = 128 * s if s > 0 else 0
                jk, ik = divmod(i, 4)
                pool, tag = pool_tag or (ps_sc, "sc")
                sc = pool.tile([128, TQC], F32, tag=tag, name="sc")
                nc.tensor.matmul(
                    sc[:, c0:TQC], kTt[jk][:, ik * 128:(ik + 1) * 128],
                    qTt[h][j][:, c0:TQC], start=True, stop=True)
                ex = est.tile([128, TQC], DT)
                nc.scalar.activation(
                    out=ex[:, c0:TQC], in_=sc[:, c0:TQC],
                    func=mybir.ActivationFunctionType.Exp, scale=SCALE)
                if s >= 0:
                    # zero the sub-diagonal triangle post-exp (cheaper than
                    # masking the scores: keeps the exp path clear)
                    nc.vector.tensor_mul(
                        ex[:, c0:c0 + 128], ex[:, c0:c0 + 128], mask01)
                exs[i] = ex

            def av(i):
                s = i - 4 * j
                # diagonal sub last: its ex also waits on affine_select
                subs = list(range(max(s, 0) + 1, 4)) + [max(s, 0)] \
                    if s >= 0 else list(range(4))
                for sub in subs:
                    cs = sub * 128
                    nc.tensor.matmul(
                        pya[sub], exs[i][:, cs:cs + 128], vt[i],
                        start=(i == 0), stop=(i == 4 * j + sub))

            scores(0)
            drain(per)
            scores(1)
            drain(per)
            for i in range(2, nblk):
                scores(i)
                av(i - 2)
                drain(per)
            av(nblk - 2)
            av(nblk - 1)
            drain(len(filler))

            # normalize: y[tq, hd] = pya[:, 0:HD] / pya[:, HD]
            ysb = ytnp.tile([128, 4, HD], DT, tag="ysb", bufs=2)
            for sub in range(4):
                rc = nrm.tile([128, 1], F32, tag="rc")
                nc.vector.reciprocal(out=rc, in_=pya[sub][:, HD:HD + 1])
                nc.vector.tensor_scalar(
                    out=ysb[:, sub, :], in0=pya[sub][:, 0:HD], scalar1=rc,
                    scalar2=None, op0=mybir.AluOpType.mult)
            return ysb

        for rep in range(reps):
            # ---- xT stream: evens on sync (xt0 split in quarters for a
            # fast PE start), odds on scalar interleaved with per-kc wq
            # chunks so Q proj j0 can track the stream ----
            # The sim (and roughly the HW) serializes DMA at aggregate BW
            # AND each dma_start costs ~1us of issue time on its queue, so:
            # few, large DMAs; only bytes needed during the stream precede
            # xt[15] (wk/wq halves lead the scalar queue, then odd chunks);
            # wv/biases/cos/sin follow (first use is post-stream), wo last.
            nc.sync.dma_start(out=xt[0][:, 0:1024], in_=xTd[0:128, 0:1024])
            nc.sync.dma_start(out=xt[0][:, 1024:T], in_=xTd[0:128, 1024:T])
            for kc in range(1, NKC):
                nc.sync.dma_start(out=xt[kc],
                                  in_=xTd[kc * 128:(kc + 1) * 128, :])
            if rep == 0:
                hkc = NKC // 2
                nc.scalar.dma_start(
                    out=wk_all[:, 0:4, :],
                    in_=wk[0:512, :].rearrange("(k p) m -> p k m", p=128))
                nc.scalar.dma_start(
                    out=wq_all[:, 0:hkc, :],
                    in_=wq[0:1024, :].rearrange("(k p) m -> p k m", p=128))
                nc.scalar.dma_start(
                    out=wk_all[:, 4:NKC, :],
                    in_=wk[512:C, :].rearrange("(k p) m -> p k m", p=128))
                nc.scalar.dma_start(
                    out=wq_all[:, hkc:NKC, :],
                    in_=wq[1024:C, :].rearrange("(k p) m -> p k m", p=128))
            if rep == 0:
                # post-stream consts: scalar queue AFTER the odd chunks so
                # the round-robin DMA service can't slot them mid-stream
                nc.scalar.dma_start(out=ident_sb, in_=idd[:, :])
                nc.scalar.dma_start(
                    out=wv_all,
                    in_=wv[:, :].rearrange("(k p) m -> p k m", p=128))
                nc.scalar.dma_start(out=bq_sb, in_=bqT[:, :])
                nc.scalar.dma_start(out=bk_sb, in_=bkT[:, :])
                nc.scalar.dma_start(
                    out=bvb_sb, in_=bass.AP(bvr, 0, [[0, 128], [1, HD]]))
                nc.scalar.dma_start(out=cos_sb, in_=cosT[:, :])
                nc.scalar.dma_start(out=sin_sb, in_=sinT[:, :])
                nc.scalar.dma_start(
                    out=wo_all,
                    in_=wo[:, :].rearrange("(h p) m -> p h m", p=128))

            # ---- wave 1: 8 PSUM groups (K j0..3 + Q h0/h1 of j0 + V
            # blocks 0/1), kc-major so the PE consumes each chunk as it
            # lands; Q h2/h3 + v2/v3 run post-stream as PE filler ----
            pk_t = [ps_sc.tile([128, TQC], F32, tag="sc", name="pk0"),
                    ps_sc.tile([128, TQC], F32, tag="sc", name="pk1"),
                    ps_a.tile([128, TQC], F32, tag="s0", name="pk2"),
                    ps_a.tile([128, TQC], F32, tag="s1", name="pk3")]
            pq_t = [ps_q.tile([128, TQC], F32, tag="pq", name="pqw0"),
                    ps_q.tile([128, TQC], F32, tag="pq", name="pqw1")]
            pv_t = [ps_a.tile([128, TQC], F32, tag="s2", name="pvw0"),
                    ps_a.tile([128, TQC], F32, tag="s3", name="pvw1")]
            # chunks arrive strictly in kc order (all on the sync queue);
            # on the last chunk, k j0 / q h0 / v0 finish FIRST since they
            # gate attention g0
            for kc in range(NKC):
                last = kc == NKC - 1
                work = [("k", 0), ("q", 0), ("v", 0), ("v", 1),
                        ("k", 1), ("q", 1), ("k", 2), ("k", 3)] if last else \
                       [("k", 0), ("k", 1), ("k", 2), ("k", 3),
                        ("q", 0), ("q", 1), ("v", 0), ("v", 1)]
                for kind, i in work:
                    if kind == "k":
                        nc.tensor.matmul(
                            pk_t[i], wk_t[kc],
                            xt[kc][:, i * TQC:(i + 1) * TQC],
                            start=(kc == 0), stop=last)
                    elif kind == "q":
                        nc.tensor.matmul(
                            pq_t[i], wq_t[kc][:, i * HD:(i + 1) * HD],
                            xt[kc][:, 0:TQC],
                            start=(kc == 0), stop=last)
                    else:
                        nc.tensor.matmul(
                            pv_t[i][:, 0:HD],
                            xt[kc][:, i * 128:i * 128 + 128], wv_t[kc],
                            start=(kc == 0), stop=last)
            # rope in two passes: the extracts free the PSUM slots fast;
            # k0/q0 finish first (they gate attention g0)
            qs_k0 = rope_extract(pk_t[0], bk_sb[:, 0:1], "qsw0")
            qs_q0 = rope_extract(pq_t[0], bq_sb[:, 0:1], "qsw1")
            nc.vector.tensor_add(vt[0][:, 0:HD], pv_t[0][:, 0:HD], bvb_sb)
            nc.vector.tensor_add(vt[1][:, 0:HD], pv_t[1][:, 0:HD], bvb_sb)
            rope_finish(qs_k0, kTt[0], 0, critical=True)
            rope_finish(qs_q0, qTt[0][0], 0, critical=True)
            qs_w = [rope_extract(pq_t[1], bq_sb[:, 1:2], "qsw2"),
                    rope_extract(pk_t[1], bk_sb[:, 0:1], "qsw3"),
                    rope_extract(pk_t[2], bk_sb[:, 0:1], "qsw4"),
                    rope_extract(pk_t[3], bk_sb[:, 0:1], "qsw5")]
            rope_finish(qs_w[0], qTt[1][0], 0)
            # k1..k3 finishes are deferred into the first attention groups
            # (kT[j] is first needed at group j*GQ) to unclog the DVE
            kfins = [(qs_w[1], kTt[1], TQC), (qs_w[2], kTt[2], 2 * TQC),
                     (qs_w[3], kTt[3], 3 * TQC)]

            # ---- attention: groups j-major; q-proj 2 ahead (h0/h1 of j0
            # came from wave 1); v blocks 2..15 and the previous group's
            # transposes are emitted BETWEEN groups (before the next pya
            # allocation so the s-tag reuse chain stays acyclic) ----
            order = [(h, j) for j in range(NTQ) for h in range(GQ)]
            ytn_all = {}
            pend = []  # (ysb, h, j, gi) awaiting transpose, deferred 2

            def oproj_unit(j, t, last=False, mid=False):
                """One 128-row output tile: po over 4 cc chunks."""
                trow = j * TQC + t * 128
                ot = outp.tile([128, C], DT)
                for cc in range(4):
                    c0 = cc * TQC
                    if mid:
                        # mid-phase units ride the pq tag (half-idle) so
                        # they never contend with pya/v/ptr on the s tags
                        po = ps_q.tile([128, TQC], F32, tag="pq", name="po")
                    else:
                        po = ps_a.tile([128, TQC], F32,
                                       tag=f"s{(t * 4 + cc) % 4}", name="po")
                    for h in range(GQ):
                        nc.tensor.matmul(
                            po, ytn_all[(h, j)][:, t * 128:(t + 1) * 128],
                            wo_t[h][:, c0:c0 + TQC],
                            start=(h == 0), stop=(h == GQ - 1))
                    if (t + cc) % 2 == 0 and not mid:
                        nc.scalar.copy(out=ot[:, c0:c0 + TQC], in_=po)
                    else:
                        nc.vector.tensor_copy(out=ot[:, c0:c0 + TQC], in_=po)
                    if last:
                        # split the final output DMA per chunk to shorten
                        # the kernel tail
                        eng = nc.sync if cc % 2 == 0 else nc.scalar
                        eng.dma_start(
                            out=yp[trow:trow + 128, c0:c0 + TQC],
                            in_=ot[:, c0:c0 + TQC])
                if not last:
                    oeng = nc.sync if mid or t % 2 == 0 else nc.scalar
                    oeng.dma_start(out=yp[trow:trow + 128, :], in_=ot)

            # o-proj (j,t) units become ready once all 4 heads of j are
            # transposed (finish_group(g_{4j+3}) at gi=4j+5); interleave
            # one per group as extra PE filler in the exp-paced region
            ounits = [(j, t) for j in range(NTQ) for t in range(4)]
            for gi, (h, j) in enumerate(order):
                if gi + 2 < len(order):
                    q_proj(*order[gi + 2])
                if len(pend) >= 1:
                    pysb, ph, pj, pgi = pend.pop(0)
                    ytn_all[(ph, pj)] = finish_group(ph, pj, pysb, pgi)
                if gi == 0:
                    v_block(2, "s0")
                    v_block(3, "s1")
                if 1 <= gi <= 3:
                    kqs, kdst, kj0 = kfins[gi - 1]
                    rope_finish(kqs, kdst, kj0)
                if 1 <= gi <= 6:
                    v_block(2 + 2 * gi, f"s{(gi + 1) % 4}")
                    v_block(3 + 2 * gi, f"s{(gi + 2) % 4}")
                ysb = attention(h, j)
                pend.append((ysb, h, j, gi))
            for pysb, ph, pj, pgi in pend:
                ytn_all[(ph, pj)] = finish_group(ph, pj, pysb, pgi)

            # ---- o-proj tail: remaining units ----
            for ui, (j, t) in enumerate(ounits):
                oproj_unit(j, t, last=(ui == len(ounits) - 1))
    _split_waits(nc, maxw=1)
    return nc


def _in_maps(x, cos, sin, Wq, bq, Wk, bk, Wv, bv, Wo):
    ident = np.eye(128, dtype=BF16)
    maps = []
    for c in range(NCORES):
        b, g = divmod(c, HK)
        qsl = slice(g * GQ * HD, (g + 1) * GQ * HD)
        ksl = slice(g * HD, (g + 1) * HD)
        maps.append({
            "xT": np.ascontiguousarray(x[b].T.astype(BF16)),
            "cosT": np.ascontiguousarray(cos[b].T.astype(BF16)),
            "sinT": np.ascontiguousarray(np.concatenate(
                [sin[b].T[64:128], -sin[b].T[0:64]], axis=0).astype(BF16)),
            "wq": np.ascontiguousarray(Wq[:, qsl].astype(BF16)),
            "wk": np.ascontiguousarray(Wk[:, ksl].astype(BF16)),
            "wv": np.ascontiguousarray(Wv[:, ksl].astype(BF16)),
            "wo": np.ascontiguousarray(Wo[qsl, :].astype(BF16)),
            "bqT": np.ascontiguousarray(
                bq[qsl].reshape(GQ, HD).T.astype(np.float32)),
            "bkT": np.ascontiguousarray(
                bk[ksl].reshape(HD, 1).astype(np.float32)),
            "bvr": np.ascontiguousarray(
                bv[ksl].reshape(1, HD).astype(np.float32)),
            "idd": ident,
        })
    return maps


_nc_cache = {}


def kernel(x, cos, sin, Wq, bq, Wk, bk, Wv, bv, Wo):
    x, cos, sin = np.asarray(x), np.asarray(cos), np.asarray(sin)
    Wq, bq = np.asarray(Wq), np.asarray(bq)
    Wk, bk = np.asarray(Wk), np.asarray(bk)
    Wv, bv = np.asarray(Wv), np.asarray(bv)
    Wo = np.asarray(Wo)
    if "nc" not in _nc_cache:
        _nc_cache["nc"] = build(reps=1)
    nc = _nc_cache["nc"]
    maps = _in_maps(x, cos, sin, Wq, bq, Wk, bk, Wv, bv, Wo)
    res = run_bass_kernel_spmd(nc, maps, core_ids=list(range(NCORES)))
    out = np.zeros((B, T, C), dtype=np.float32)
    for c in range(NCORES):
        b = c // HK
        out[b] += res.results[c]["yp"].astype(np.float32)
    return out
